# revision 7
# baseline (speedup 1.0000x reference)
"""Expert-parallel MoE (Mixtral-style top-2 of 8 experts, SwiGLU) on 8 TRN2 cores.

Strategy: one expert per NeuronCore. Routing (softmax/top-k/renorm) is tiny
(1024x8) and runs on host during input sharding; each core runs a dense
SwiGLU MLP over only the tokens routed to its expert (~256 of 1024, padded to
a common capacity C), with the renormalized routing weight folded in on
device. Host scatter-adds the per-expert outputs back to [T, H].

Device kernel per core (all matmuls bf16, fp32 PSUM accumulation):
  phase B: G[i, t]   = W13T[h, i].T-accum over h of x[t, h]   (gate+up halves)
           act[i, t] = silu(G_gate) * G_up                    (bf16 in SBUF)
  phase C: y[t, hh]  = sum_i act[i, t].T @ W2T[i, hh], scaled by routing w[t]

Weights are pre-transposed/tiled/bf16-cast on host so every device DMA is
fully contiguous per partition.
"""

import os

import ml_dtypes
import numpy as np

import concourse.bass as bass
from concourse import bacc
import concourse.mybir as mybir
import concourse.tile as tile
from concourse.bass_utils import run_bass_kernel_spmd

P = 128
H = 2048          # hidden dim
INTER = 4096      # intermediate dim
E = 8             # experts == cores
N_CORES = 8
HCHUNK = 512      # output column chunk (one PSUM bank of fp32)
BF16 = mybir.dt.bfloat16
F32 = mybir.dt.float32

KO = H // P           # 16 contraction steps over hidden dim
NJ = INTER // P       # 32 tiles over intermediate dim
HC = H // HCHUNK      # 4 output column chunks

# set by kernel() for test harness introspection
last_results = None


def _build_nc(C: int) -> bass.Bass:
    act_fn = mybir.ActivationFunctionType

    nc = bacc.Bacc()
    xt_d = nc.declare_dram_parameter("xt", [P, KO, C], BF16, isOutput=False)
    w13_d = nc.declare_dram_parameter("w13", [2 * NJ, P, KO, P], BF16, isOutput=False)
    w2_d = nc.declare_dram_parameter("w2", [NJ, P, H], BF16, isOutput=False)
    y_d = nc.declare_dram_parameter("y", [H, C], F32, isOutput=True)

    with tile.TileContext(nc) as tc:
        with (
            tc.tile_pool(name="xp", bufs=1) as xp,
            tc.tile_pool(name="w13p", bufs=4) as w13p,
            tc.tile_pool(name="w2p", bufs=1) as w2p,
            tc.tile_pool(name="actp", bufs=1) as actp,
            tc.tile_pool(name="silup", bufs=2) as silup,
            tc.tile_pool(name="outp", bufs=3) as outp,
            tc.tile_pool(name="ps13", bufs=2, space="PSUM") as ps13,
            tc.tile_pool(name="psy", bufs=2, space="PSUM") as psy,
        ):
            # first gate-weight tile loads ahead of everything so PE can
            # start ~9us in; xt slices are independent tiles so their DMAs
            # fan out across queues instead of serializing on a WAW chain
            w13_first = w13p.tile([P, KO, P], BF16, tag="w13")
            nc.sync.dma_start(w13_first[:], w13_d[0])
            xt_tiles = []
            for ko in range(KO):
                xt_sb = xp.tile([P, C], BF16, tag=f"xt_{ko}")
                nc.sync.dma_start(xt_sb[:], xt_d[:, ko, :])
                xt_tiles.append(xt_sb)

            # phase B: gate/up projections + SwiGLU, one 128-wide i-tile at a
            # time; w2 tiles are prefetched on spare DMA bandwidth as we go
            w2_tiles = []
            act_tiles = []
            for j in range(NJ):
                ps_pair = []
                for k in (j, NJ + j):  # gate half, up half of W13
                    if k == 0:
                        w13_sb = w13_first
                    else:
                        w13_sb = w13p.tile([P, KO, P], BF16, tag="w13")
                        nc.sync.dma_start(w13_sb[:], w13_d[k])
                    ps = ps13.tile([P, C], F32, tag="g" if k == j else "u")
                    for ko in range(KO):
                        nc.tensor.matmul(
                            ps[:],
                            w13_sb[:, ko, :],
                            xt_tiles[ko][:],
                            start=(ko == 0),
                            stop=(ko == KO - 1),
                        )
                    ps_pair.append(ps)
                g_ps, u_ps = ps_pair

                w2_sb = w2p.tile([P, H], BF16, tag=f"w2_{j}")
                nc.sync.dma_start(w2_sb[:], w2_d[j])
                w2_tiles.append(w2_sb)

                # silu(g)*u as sigmoid + 2 muls (CoreSim has no Silu)
                s_sb = silup.tile([P, C], F32, tag="silu")
                nc.scalar.activation(s_sb[:], g_ps[:], act_fn.Sigmoid)
                su_sb = silup.tile([P, C], F32, tag="su")
                nc.vector.tensor_mul(su_sb[:], s_sb[:], u_ps[:])
                a_sb = actp.tile([P, C], BF16, tag=f"act_{j}")
                nc.vector.tensor_mul(a_sb[:], su_sb[:], g_ps[:])
                act_tiles.append(a_sb)

            # phase C: down projection, output [H, C]; routing-weight scale
            # happens host-side during the scatter-add
            for hh in range(H // P):
                y_ps = psy.tile([P, C], F32, tag="y")
                for j in range(NJ):
                    nc.tensor.matmul(
                        y_ps[:],
                        w2_tiles[j][:, hh * P:(hh + 1) * P],
                        act_tiles[j][:],
                        start=(j == 0),
                        stop=(j == NJ - 1),
                    )
                o_sb = outp.tile([P, C], F32, tag="o")
                nc.vector.tensor_copy(o_sb[:], y_ps[:])
                nc.sync.dma_start(y_d[hh * P:(hh + 1) * P, :], o_sb[:])
    nc.compile()
    return nc


def _route(router_logits: np.ndarray, top_k: int):
    """Match jax.nn.softmax + jax.lax.top_k + renormalize (ties -> lower idx)."""
    p = router_logits.astype(np.float64)
    p = np.exp(p - p.max(axis=-1, keepdims=True))
    p /= p.sum(axis=-1, keepdims=True)
    order = np.argsort(-p, axis=-1, kind="stable")
    idx = order[:, :top_k]
    w = np.take_along_axis(p, idx, axis=-1)
    w /= w.sum(axis=-1, keepdims=True)
    return idx, w


def kernel(hidden_states, router_logits, W13, W2, top_k):
    global last_results
    top_k = int(top_k)
    hs = np.asarray(hidden_states, dtype=np.float32)
    T = hs.shape[0]
    idx, w = _route(np.asarray(router_logits, dtype=np.float32), top_k)

    tok_ids, tok_w = [], []
    for e in range(E):
        sel = idx == e  # [T, k]; at most one True per row
        rows = np.nonzero(sel.any(axis=-1))[0]
        tok_ids.append(rows)
        tok_w.append(w[sel].astype(np.float32))  # row-major -> token order

    C = max(16, -(-max(len(r) for r in tok_ids) // 16) * 16)

    W13 = np.asarray(W13, dtype=np.float32)
    W2 = np.asarray(W2, dtype=np.float32)
    in_maps = []
    for e in range(E):
        rows = tok_ids[e]
        n_e = len(rows)
        xt = np.zeros((P, KO, C), dtype=ml_dtypes.bfloat16)
        if n_e:
            xg = hs[rows].astype(ml_dtypes.bfloat16)  # [n_e, H]
            xt[:, :, :n_e] = xg.T.reshape(KO, P, n_e).transpose(1, 0, 2)
        w13 = np.ascontiguousarray(
            W13[e].astype(ml_dtypes.bfloat16)
            .reshape(2 * NJ, P, KO, P).transpose(0, 3, 2, 1)
        )
        w2 = np.ascontiguousarray(
            W2[e].astype(ml_dtypes.bfloat16)
            .reshape(H, NJ, P).transpose(1, 2, 0)
        )
        in_maps.append({"xt": xt, "w13": w13, "w2": w2})

    nc = _build_nc(C)
    res = run_bass_kernel_spmd(
        nc,
        in_maps,
        list(range(N_CORES)),
        trace=bool(os.environ.get("MOE_TRACE")),
        tmpdir=os.environ.get("MOE_TRACE_DIR") or None,
    )
    last_results = res

    out = np.zeros((T, H), dtype=np.float32)
    for e in range(E):
        rows = tok_ids[e]
        n_e = len(rows)
        if n_e:
            y = res.results[e]["y"]  # [H, C]
            out[rows] += y[:, :n_e].T * tok_w[e][:, None]
    return out


# revision 9
# speedup vs baseline: 1.0329x; 1.0329x over previous
"""Expert-parallel MoE (Mixtral-style top-2 of 8 experts, SwiGLU) on 8 TRN2 cores.

Strategy: one expert per NeuronCore. Routing (softmax/top-k/renorm) is tiny
(1024x8) and runs on host during input sharding; each core runs a dense
SwiGLU MLP over only the tokens routed to its expert (~256 of 1024, padded to
a common capacity C), with the renormalized routing weight folded in on
device. Host scatter-adds the per-expert outputs back to [T, H].

Device kernel per core (all matmuls bf16, fp32 PSUM accumulation):
  phase B: G[i, t]   = W13T[h, i].T-accum over h of x[t, h]   (gate+up halves)
           act[i, t] = silu(G_gate) * G_up                    (bf16 in SBUF)
  phase C: y[t, hh]  = sum_i act[i, t].T @ W2T[i, hh], scaled by routing w[t]

Weights are pre-transposed/tiled/bf16-cast on host so every device DMA is
fully contiguous per partition.
"""

import os

import ml_dtypes
import numpy as np

import concourse.bass as bass
from concourse import bacc
import concourse.mybir as mybir
import concourse.tile as tile
from concourse.bass_utils import run_bass_kernel_spmd

P = 128
H = 2048          # hidden dim
INTER = 4096      # intermediate dim
E = 8             # experts == cores
N_CORES = 8
HCHUNK = 512      # output column chunk (one PSUM bank of fp32)
BF16 = mybir.dt.bfloat16
F32 = mybir.dt.float32

KO = H // P           # 16 contraction steps over hidden dim
NJ = INTER // P       # 32 tiles over intermediate dim
HC = H // HCHUNK      # 4 output column chunks
NC1 = 4               # down-proj column tiles interleaved into phase B

# set by kernel() for test harness introspection
last_results = None


def _build_nc(C: int) -> bass.Bass:
    act_fn = mybir.ActivationFunctionType

    nc = bacc.Bacc()
    xt_d = nc.declare_dram_parameter("xt", [P, KO, C], BF16, isOutput=False)
    w13_d = nc.declare_dram_parameter("w13", [2 * NJ, P, KO, P], BF16, isOutput=False)
    w2_d = nc.declare_dram_parameter("w2", [NJ, P, H], BF16, isOutput=False)
    y_d = nc.declare_dram_parameter("y", [H, C], F32, isOutput=True)

    with tile.TileContext(nc) as tc:
        with (
            tc.tile_pool(name="xp", bufs=1) as xp,
            tc.tile_pool(name="w13p", bufs=4) as w13p,
            tc.tile_pool(name="w2p", bufs=1) as w2p,
            tc.tile_pool(name="actp", bufs=1) as actp,
            tc.tile_pool(name="silup", bufs=2) as silup,
            tc.tile_pool(name="outp", bufs=3) as outp,
            tc.tile_pool(name="ps13", bufs=2, space="PSUM") as ps13,
            tc.tile_pool(name="psy", bufs=2, space="PSUM") as psy,
        ):
            # first gate-weight tile loads ahead of everything so PE can
            # start ~9us in; xt slices are independent tiles so their DMAs
            # fan out across queues instead of serializing on a WAW chain
            w13_first = w13p.tile([P, KO, P], BF16, tag="w13")
            nc.sync.dma_start(w13_first[:], w13_d[0])
            xt_tiles = []
            for ko in range(KO):
                xt_sb = xp.tile([P, C], BF16, tag=f"xt_{ko}")
                nc.sync.dma_start(xt_sb[:], xt_d[:, ko, :])
                xt_tiles.append(xt_sb)

            # phase B: gate/up projections + SwiGLU, one 128-wide i-tile at a
            # time; w2 tiles are prefetched on spare DMA bandwidth as we go
            w2_tiles = []
            act_tiles = []
            c1_ps = []
            for i in range(NC1):
                c1 = psy.tile([P, C], F32, tag="y", name=f"c1_{i}")
                c1_ps.append(c1)
            for j in range(NJ):
                ps_pair = []
                for k in (j, NJ + j):  # gate half, up half of W13
                    if k == 0:
                        w13_sb = w13_first
                    else:
                        w13_sb = w13p.tile([P, KO, P], BF16, tag="w13")
                        nc.sync.dma_start(w13_sb[:], w13_d[k])
                    ps = ps13.tile([P, C], F32, tag="g" if k == j else "u")
                    for ko in range(KO):
                        nc.tensor.matmul(
                            ps[:],
                            w13_sb[:, ko, :],
                            xt_tiles[ko][:],
                            start=(ko == 0),
                            stop=(ko == KO - 1),
                        )
                    ps_pair.append(ps)
                g_ps, u_ps = ps_pair

                w2_sb = w2p.tile([P, H], BF16, tag=f"w2_{j}")
                nc.sync.dma_start(w2_sb[:], w2_d[j])
                w2_tiles.append(w2_sb)

                # silu(g)*u as sigmoid + 2 muls (CoreSim has no Silu)
                s_sb = silup.tile([P, C], F32, tag="silu")
                nc.scalar.activation(s_sb[:], g_ps[:], act_fn.Sigmoid)
                su_sb = silup.tile([P, C], F32, tag="su")
                nc.vector.tensor_mul(su_sb[:], s_sb[:], u_ps[:])
                a_sb = actp.tile([P, C], BF16, tag=f"act_{j}")
                nc.vector.tensor_mul(a_sb[:], su_sb[:], g_ps[:])
                act_tiles.append(a_sb)

                # interleave the first NC1 down-proj column tiles into phase
                # B (lagging one j so ACT/DVE have time to produce act) --
                # keeps PE fed while the weight DMA stream catches up
                if j >= 1:
                    for hh in range(NC1):
                        nc.tensor.matmul(
                            c1_ps[hh][:],
                            w2_tiles[j - 1][:, hh * P:(hh + 1) * P],
                            act_tiles[j - 1][:],
                            start=(j - 1 == 0),
                            stop=False,
                        )

            def emit_down(y_ps, hh, j0, j1, start, stop):
                for j in range(j0, j1):
                    nc.tensor.matmul(
                        y_ps[:],
                        w2_tiles[j][:, hh * P:(hh + 1) * P],
                        act_tiles[j][:],
                        start=start and (j == j0),
                        stop=stop and (j == j1 - 1),
                    )

            def writeback(y_ps, hh):
                o_sb = outp.tile([P, C], F32, tag="o")
                nc.vector.tensor_copy(o_sb[:], y_ps[:])
                nc.sync.dma_start(y_d[hh * P:(hh + 1) * P, :], o_sb[:])

            # finish the interleaved accumulators (last j), then drain
            for hh in range(NC1):
                emit_down(c1_ps[hh], hh, NJ - 1, NJ, start=False, stop=True)
                writeback(c1_ps[hh], hh)

            # phase C2: remaining down-proj column tiles
            for hh in range(NC1, H // P):
                y_ps = psy.tile([P, C], F32, tag="y")
                emit_down(y_ps, hh, 0, NJ, start=True, stop=True)
                writeback(y_ps, hh)
    nc.compile()
    return nc


def _route(router_logits: np.ndarray, top_k: int):
    """Match jax.nn.softmax + jax.lax.top_k + renormalize (ties -> lower idx)."""
    p = router_logits.astype(np.float64)
    p = np.exp(p - p.max(axis=-1, keepdims=True))
    p /= p.sum(axis=-1, keepdims=True)
    order = np.argsort(-p, axis=-1, kind="stable")
    idx = order[:, :top_k]
    w = np.take_along_axis(p, idx, axis=-1)
    w /= w.sum(axis=-1, keepdims=True)
    return idx, w


def kernel(hidden_states, router_logits, W13, W2, top_k):
    global last_results
    top_k = int(top_k)
    hs = np.asarray(hidden_states, dtype=np.float32)
    T = hs.shape[0]
    idx, w = _route(np.asarray(router_logits, dtype=np.float32), top_k)

    tok_ids, tok_w = [], []
    for e in range(E):
        sel = idx == e  # [T, k]; at most one True per row
        rows = np.nonzero(sel.any(axis=-1))[0]
        tok_ids.append(rows)
        tok_w.append(w[sel].astype(np.float32))  # row-major -> token order

    C = max(16, -(-max(len(r) for r in tok_ids) // 16) * 16)

    W13 = np.asarray(W13, dtype=np.float32)
    W2 = np.asarray(W2, dtype=np.float32)
    in_maps = []
    for e in range(E):
        rows = tok_ids[e]
        n_e = len(rows)
        xt = np.zeros((P, KO, C), dtype=ml_dtypes.bfloat16)
        if n_e:
            xg = hs[rows].astype(ml_dtypes.bfloat16)  # [n_e, H]
            xt[:, :, :n_e] = xg.T.reshape(KO, P, n_e).transpose(1, 0, 2)
        w13 = np.ascontiguousarray(
            W13[e].astype(ml_dtypes.bfloat16)
            .reshape(2 * NJ, P, KO, P).transpose(0, 3, 2, 1)
        )
        w2 = np.ascontiguousarray(
            W2[e].astype(ml_dtypes.bfloat16)
            .reshape(H, NJ, P).transpose(1, 2, 0)
        )
        in_maps.append({"xt": xt, "w13": w13, "w2": w2})

    nc = _build_nc(C)
    res = run_bass_kernel_spmd(
        nc,
        in_maps,
        list(range(N_CORES)),
        trace=bool(os.environ.get("MOE_TRACE")),
        tmpdir=os.environ.get("MOE_TRACE_DIR") or None,
    )
    last_results = res

    out = np.zeros((T, H), dtype=np.float32)
    for e in range(E):
        rows = tok_ids[e]
        n_e = len(rows)
        if n_e:
            y = res.results[e]["y"]  # [H, C]
            out[rows] += y[:, :n_e].T * tok_w[e][:, None]
    return out
